# revision 3
# baseline (speedup 1.0000x reference)
"""Trainium2 Bass kernel for a pre-LN multi-head attention block (v3).

Computes, for x of shape (4, 2048, 512):
    xn  = LayerNorm(x) * gamma + beta
    q/k/v = xn @ W{q,k,v}.T + b{q,k,v}          (8 heads, dk=64)
    attn  = softmax(q k^T / sqrt(dk)) @ v
    out   = attn @ Wo.T + bo
Sharding: 8 cores = (4 batches) x (2 query-halves); per-core outputs are
disjoint row blocks, host gather is concatenation.

Redesign vs the 260-270us exp-paced baseline (~205us measured):
  - Softmax exp is split across TWO engines: half the key-tiles use
    ScalarE ACT exp (~1.1us per [128,1024] tile), the other half run on
    the otherwise-underused VectorE via the Schraudolph bit trick -- one
    fused tensor_scalar (x*C1 + C2) converted f32->int16 (RNE on HW)
    whose bit pattern IS bf16 exp(x*scale) (~1.0us/tile, ~3% pointwise
    noise that averages out over the 2048-key softmax; with half the
    tiles on DVE the end-to-end error is ~1.2e-2 < 2e-2).
  - The attention loop processes HEAD PAIRS: kt/qt chunk hp holds head
    2hp on partitions 0-63 and head 2hp+1 on 64-127; the two scores
    matmuls (contraction dk=64) sit on disjoint PE row groups
    (tile_position (0,0)/(64,0), auto-derived) and co-issue when the
    first is not semaphore-blocked; the two PV matmuls likewise co-issue.
  - 8 phases of (head-pair, query-half) x 16 key-tile units.  PSUM in
    the loop is exactly 8 banks: score pair-tiles [128,1024]f32 x3
    (6 banks, triple-buffered so exp-latency jitter cannot stall the
    score matmuls and break matmul-pair co-issue) + 2 PV accumulators
    [66,512]f32.  Per-phase norm: ACT drains [o|sums] to SBUF, a
    partition-64->0 copy feeds reciprocal_approx_fast (the custom DVE op
    requires a partition-0 SBUF input), gpsimd broadcast, DVE multiply;
    the two drains fire at the phase seam ahead of the ACT exp backlog.
  - All q/k/v projections + LayerNorm run in a PE-dense preamble whose
    emission order matches input-arrival order (x tiles 0-7, weights,
    x tiles 8-15) -- every engine FIFO sees work in readiness order.
    The ACT exp table is pre-warmed after the last LN Sqrt so the loop's
    first exp pays no ~2.7us table switch.
  - K-projection bias is dropped entirely: exp(q.(k+bk)) contributes a
    per-query constant factor exp(q.bk) that cancels in softmax
    normalization.  (The q bias survives; v bias is folded into bo.)
"""

import ml_dtypes
import numpy as np

import concourse.bass as bass
import concourse.mybir as mybir
import concourse.tile as tile
from concourse import bacc
from concourse.bass_utils import run_bass_kernel_spmd
from concourse.masks import make_identity

F32 = mybir.dt.float32
BF16 = mybir.dt.bfloat16
I16 = mybir.dt.int16
ALU = mybir.AluOpType
ACTF = mybir.ActivationFunctionType

P = 128          # partitions
DIM = 512        # model dim
H = 8            # heads
DK = 64          # head dim
NTOK = 2048      # tokens per core (one batch's sequence)
NQ = 1024        # queries per core (half the sequence)
CC = DIM // P    # 4 contraction chunks of 128 (also the head-pair index)
TT = NTOK // P   # 16 token tiles
JT = NTOK // P   # 16 key tiles
NB = 512         # moving-operand limit per matmul
EPS = 1e-5
SCALE = DK ** -0.5

# Schraudolph exp-as-int-bits constants: int16 bits of bf16 exp(SCALE*x)
#   bits = round(x * SCALE * 128/ln2 + (127*128 - 5.59))
# (+0.5 so a truncating f32->int16 convert behaves like round; if the HW
# rounds, the extra half-LSB is a common-mode factor softmax cancels).
SCH_C1 = SCALE * 128.0 / float(np.log(2.0))
SCH_C2 = 127.0 * 128.0 - 5.59 + 0.5

# Key-tile slots (of 16 per phase) whose exp runs on VectorE instead of
# ScalarE.  8/16 balances the two engines' loop-time budgets (ACT also
# carries the two phase-seam drains); even slots so the norm-chain
# thunks (popped on odd units) land where DVE is free.
DVE_SLOTS = frozenset((0, 2, 4, 6, 8, 10, 12, 14))

N_CORES = 8
_BUILT = None


def _build():
    nc = bacc.Bacc("TRN2", target_bir_lowering=False, debug=False,
                   num_devices=N_CORES)

    xq = nc.dram_tensor("xq", [NTOK, DIM], F32, kind="ExternalInput")
    wqT = nc.dram_tensor("wqT", [DIM, DIM], BF16, kind="ExternalInput")
    wkT = nc.dram_tensor("wkT", [DIM, DIM], BF16, kind="ExternalInput")
    wvT = nc.dram_tensor("wvT", [DIM, DIM], BF16, kind="ExternalInput")
    woT = nc.dram_tensor("woT", [DK, H, DIM], BF16, kind="ExternalInput")
    qb_c = nc.dram_tensor("qb_c", [P, CC], F32, kind="ExternalInput")
    bo_b = nc.dram_tensor("bo_b", [P, DIM], F32, kind="ExternalInput")
    y = nc.dram_tensor("y", [NQ, DIM], F32, kind="ExternalOutput")

    with tile.TileContext(nc) as tc:
        with (
            tc.tile_pool(name="const", bufs=1) as const,
            tc.tile_pool(name="persist", bufs=1) as persist,
            tc.tile_pool(name="lnp", bufs=8) as lnp,
            tc.tile_pool(name="stp", bufs=8) as stp,
            tc.tile_pool(name="epp", bufs=4) as epp,
            tc.tile_pool(name="otp", bufs=3) as otp,
            tc.tile_pool(name="rpp", bufs=4) as rpp,
            tc.tile_pool(name="outp", bufs=3) as outp,
            # PSUM: spp 3x[128,1024]f32 (6 banks) + opp 2x 1-bank = 8
            tc.tile_pool(name="spp", bufs=3, space="PSUM") as spp,
            tc.tile_pool(name="opp", bufs=2, space="PSUM") as opp,
        ):
            ident = const.tile([P, P], BF16)
            make_identity(nc, ident)
            # DMA order = consumption order: first 8 x-tiles (LN stats lead),
            # then projection weights, then the rest of x.
            xts = []
            for tt in range(8):
                xt = lnp.tile([P, DIM], F32, tag="xt", name=f"xt{tt}")
                nc.sync.dma_start(out=xt, in_=xq.ap()[tt * P:(tt + 1) * P, :])
                xts.append(xt)
            wv = const.tile([P, CC, DIM], BF16)
            nc.sync.dma_start(out=wv, in_=wvT.ap().rearrange(
                "(cc p) d -> p cc d", p=P))
            wq = const.tile([P, CC, DIM], BF16)
            nc.sync.dma_start(out=wq, in_=wqT.ap().rearrange(
                "(cc p) d -> p cc d", p=P))
            wk = const.tile([P, CC, DIM], BF16)
            nc.sync.dma_start(out=wk, in_=wkT.ap().rearrange(
                "(cc p) d -> p cc d", p=P))
            qb = const.tile([P, CC], F32)
            nc.sync.dma_start(out=qb, in_=qb_c.ap())
            for tt in range(8, TT):
                xt = lnp.tile([P, DIM], F32, tag="xt", name=f"xt{tt}")
                nc.sync.dma_start(out=xt, in_=xq.ap()[tt * P:(tt + 1) * P, :])
                xts.append(xt)

            bob = const.tile([P, DIM], F32)
            nc.sync.dma_start(out=bob, in_=bo_b.ap())
            epst = const.tile([P, 1], F32)
            nc.vector.memset(epst, EPS)
            wo = const.tile([DK, H, DIM], BF16)
            nc.sync.dma_start(out=wo, in_=woT.ap())

            # Persistent activations.
            xnT = persist.tile([P, CC, NTOK], BF16)    # xn^T
            qt = persist.tile([P, CC, NQ], BF16)       # Q^T
            kt = persist.tile([P, CC, NTOK], BF16)     # K^T
            vp = persist.tile([P, JT, H, DK + 2], BF16)  # [V_h | 1 | 0]
            onT = persist.tile([DK, H, NQ], BF16)      # normalized O^T

            nc.vector.memset(vp[:, :, :, DK], 1.0)
            nc.vector.memset(vp[:, :, :, DK + 1], 0.0)

            # ---- projection helpers -------------------------------------
            def v_proj(j):
                ps = opp.tile([P, DIM], F32, tag="ops", name=f"v{j}")
                for cc in range(CC):
                    nc.tensor.matmul(ps, lhsT=xnT[:, cc, j * P:(j + 1) * P],
                                     rhs=wv[:, cc, :],
                                     start=(cc == 0), stop=(cc == CC - 1))
                nc.scalar.activation(
                    out=vp[:, j, :, 0:DK],
                    in_=ps.rearrange("p (h d) -> p h d", d=DK),
                    func=ACTF.Copy)

            def q_chunk(t, ib):
                ps = spp.tile([P, NB], F32, tag="sp", name=f"q{t}_{ib}")
                for cc in range(CC):
                    nc.tensor.matmul(ps, lhsT=wq[:, cc, t * P:(t + 1) * P],
                                     rhs=xnT[:, cc, ib * NB:(ib + 1) * NB],
                                     start=(cc == 0), stop=(cc == CC - 1))
                nc.vector.tensor_scalar(
                    out=qt[:, t, ib * NB:(ib + 1) * NB], in0=ps,
                    scalar1=qb[:, t:t + 1], scalar2=None, op0=ALU.add)

            def k_chunk(t, ib):
                # no bias: exp(q.bk) is constant across keys -> cancels in
                # softmax normalization.
                ps = spp.tile([P, NB], F32, tag="sp", name=f"k{t}_{ib}")
                for cc in range(CC):
                    nc.tensor.matmul(ps, lhsT=wk[:, cc, t * P:(t + 1) * P],
                                     rhs=xnT[:, cc, ib * NB:(ib + 1) * NB],
                                     start=(cc == 0), stop=(cc == CC - 1))
                nc.scalar.activation(out=kt[:, t, ib * NB:(ib + 1) * NB],
                                     in_=ps, func=ACTF.Copy)

            # ---- LayerNorm + transpose + projections --------------------
            G = 4  # stats group: batch the sqrt+reciprocal across 4 tiles
            mvg = [persist.tile([P, G, 2], F32, name=f"mvg{g}")
                   for g in range(TT // G)]
            rsg = [persist.tile([P, G], F32, name=f"rsg{g}")
                   for g in range(TT // G)]

            def ln_stats(tt):
                g, gi = divmod(tt, G)
                stats = stp.tile([P, 6], F32)
                nc.vector.bn_stats(out=stats, in_=xts[tt])
                nc.vector.bn_aggr(out=mvg[g][:, gi, :], in_=stats)
                if gi == G - 1:
                    nc.scalar.activation(out=rsg[g], in_=mvg[g][:, :, 1],
                                         func=ACTF.Sqrt, bias=epst)
                    nc.vector.reciprocal(out=rsg[g], in_=rsg[g])

            def ln_chain(tt):
                # z-scale, transpose, and xn^T evacuation for tile tt
                g, gi = divmod(tt, G)
                z = lnp.tile([P, DIM], BF16, tag="z", name=f"z{tt}")
                nc.vector.tensor_scalar(out=z, in0=xts[tt],
                                        scalar1=mvg[g][:, gi, 0:1],
                                        scalar2=rsg[g][:, gi:gi + 1],
                                        op0=ALU.subtract, op1=ALU.mult)
                zt4 = spp.tile([P, DIM], BF16, tag="sp", name=f"zt{tt}")
                for cc in range(CC):
                    nc.tensor.transpose(zt4[:, cc * P:(cc + 1) * P],
                                        z[:, cc * P:(cc + 1) * P], ident)
                nc.scalar.activation(
                    out=xnT[:, :, tt * P:(tt + 1) * P],
                    in_=zt4.rearrange("p (cc q) -> p cc q", cc=CC),
                    func=ACTF.Copy)

            # Explicit readiness-ordered emission: every engine's FIFO sees
            # work in roughly the order its inputs (DMA, stats, weights)
            # arrive, so ready work never sits behind a blocked instruction.
            for tt in range(4):
                ln_stats(tt)
            for tt in range(4):
                ln_chain(tt)
            for tt in range(4, 8):
                ln_stats(tt)
            for tt in range(4):
                v_proj(tt)
            for tt in range(4, 8):
                ln_chain(tt)
            for tt in range(4, 7):
                v_proj(tt)
            for ib in range(2):
                for t in range(CC):
                    q_chunk(t, ib)
            for ib in range(2):
                for t in range(CC):
                    k_chunk(t, ib)
            for tt in range(8, 12):
                ln_stats(tt)
            for tt in range(8, 12):
                ln_chain(tt)
            for tt in range(7, 11):
                v_proj(tt)
            for tt in range(12, 16):
                ln_stats(tt)
            # Warm the ACT exp table set now (after the last Sqrt) so the
            # attention loop's first exp doesn't pay the ~2.7us table load.
            expwarm = stp.tile([1, 1], F32)
            nc.scalar.activation(out=expwarm, in_=epst[0:1, 0:1],
                                 func=ACTF.Exp)
            for t in range(CC):
                k_chunk(t, 2)
            for tt in range(12, 16):
                ln_chain(tt)
            for tt in range(11, 16):
                v_proj(tt)
            for t in range(CC):
                k_chunk(t, 3)

            # ---- attention: 8 phases of (head-pair hp, query-half m) ----
            # Per phase: 16 key-tile units.  Unit = 2 concurrent row-tiled
            # score matmuls -> exp (ACT or DVE) -> 2 PV matmuls (emitted
            # one unit behind so the PE never waits inline on the exp).
            pend = None
            deferred = []
            ops_by_phase = {}

            def emit_pv(p_, j_, etb):
                hp_ = p_ // 2
                if j_ == 0:
                    ops_by_phase[p_] = (
                        opp.tile([DK + 2, NB], F32, tag="ops",
                                 name=f"o{p_}a"),
                        opp.tile([DK + 2, NB], F32, tag="ops",
                                 name=f"o{p_}b"),
                    )
                oa, ob = ops_by_phase[p_]
                nc.tensor.matmul(oa, lhsT=vp[:, j_, 2 * hp_, :],
                                 rhs=etb[:, 0:NB],
                                 start=(j_ == 0), stop=(j_ == JT - 1))
                nc.tensor.matmul(ob, lhsT=vp[:, j_, 2 * hp_ + 1, :],
                                 rhs=etb[:, NB:2 * NB],
                                 start=(j_ == 0), stop=(j_ == JT - 1))

            def norm_thunks(p_):
                # softmax normalization for phase p_ = (hp, m): both heads'
                # 512-query slices.  ACT drains [o | sums] to SBUF (custom
                # DVE ops can't read PSUM), then reciprocal_approx_fast of
                # the sums row, gpsimd partition-broadcast, multiply.
                hp_, m_ = divmod(p_, 2)
                q0 = m_ * NB
                oa, ob = ops_by_phase.pop(p_)
                out = []
                for hx, op_t in ((2 * hp_, oa), (2 * hp_ + 1, ob)):
                    ot = otp.tile([DK + 1, NB], F32, tag="ot",
                                  name=f"ot{p_}_{hx}")
                    s0 = rpp.tile([1, NB], F32, tag="s0",
                                  name=f"s{p_}_{hx}")
                    rinv = rpp.tile([1, NB], F32, tag="r",
                                    name=f"r{p_}_{hx}")
                    rb = rpp.tile([DK, NB], F32, tag="rb",
                                  name=f"rb{p_}_{hx}")

                    def drain(ot=ot, op_t=op_t):
                        nc.scalar.activation(out=ot, in_=op_t[0:DK + 1, :],
                                             func=ACTF.Copy)

                    def scopy(s0=s0, ot=ot):
                        # sums row partition 64 -> 0 (reciprocal_approx_fast
                        # requires a partition-0 SBUF input)
                        nc.vector.tensor_copy(out=s0, in_=ot[DK:DK + 1, :])

                    def recip(rinv=rinv, s0=s0):
                        nc.vector.reciprocal_approx_fast(out=rinv, in_=s0)

                    def bcast(rb=rb, rinv=rinv):
                        nc.gpsimd.partition_broadcast(rb, rinv)

                    def mult(hx=hx, ot=ot, rb=rb, q0=q0):
                        nc.vector.tensor_tensor(
                            out=onT[:, hx, q0:q0 + NB], in0=ot[0:DK, :],
                            in1=rb, op=ALU.mult)
                    out.extend((drain, scopy, recip, bcast, mult))
                # both drains first (fired immediately at the phase seam,
                # ahead of the new phase's ACT exps), then copies/recips,
                # then bcasts, then mults
                return [out[0], out[5], out[1], out[2], out[6], out[7],
                        out[3], out[8], out[4], out[9]]

            for p in range(2 * CC):
                hp, m = divmod(p, 2)
                q0 = m * NB
                for j in range(JT):
                    sp = spp.tile([P, 2 * NB], F32, tag="sp",
                                  name=f"sp{p}_{j}")
                    jb = slice(j * P, (j + 1) * P)
                    qs = slice(q0, q0 + NB)
                    nc.tensor.matmul(sp[:, 0:NB], lhsT=kt[0:DK, hp, jb],
                                     rhs=qt[0:DK, hp, qs],
                                     start=True, stop=True)
                    nc.tensor.matmul(sp[:, NB:2 * NB], lhsT=kt[DK:P, hp, jb],
                                     rhs=qt[DK:P, hp, qs],
                                     start=True, stop=True)
                    if j in DVE_SLOTS:
                        eti = epp.tile([P, 2 * NB], I16, tag="et",
                                       name=f"e{p}_{j}")
                        nc.vector.tensor_scalar(
                            out=eti, in0=sp, scalar1=SCH_C1, scalar2=SCH_C2,
                            op0=ALU.mult, op1=ALU.add)
                        etb = eti.bitcast(BF16)
                    else:
                        etb = epp.tile([P, 2 * NB], BF16, tag="et",
                                       name=f"e{p}_{j}")
                        nc.scalar.activation(out=etb, in_=sp, func=ACTF.Exp,
                                             scale=SCALE)
                    if pend is not None:
                        emit_pv(*pend)
                        if pend[1] == JT - 1:
                            th = norm_thunks(pend[0])
                            th[0]()   # drain_a -- queue ahead of this
                            th[1]()   # drain_b -- phase's ACT exp backlog
                            deferred.extend(th[2:])
                    pend = (p, j, etb)
                    if deferred and j % 2 == 1:
                        deferred.pop(0)()

            # ---- tail ---------------------------------------------------
            # Final PV, then the last phase's norm chain overlapped with
            # y_proj on the first 4 query-tiles (their heads are done).
            emit_pv(*pend)
            tailn = norm_thunks(pend[0])
            for t in tailn[:8]:
                t()
            tailn = tailn[8:]

            def y_proj(it):
                yps = spp.tile([P, DIM], F32, tag="sp", name=f"y{it}")
                for h in range(H):
                    nc.tensor.matmul(
                        yps, lhsT=onT[:, h, it * P:(it + 1) * P],
                        rhs=wo[:, h, :],
                        start=(h == 0), stop=(h == H - 1))
                yo = outp.tile([P, DIM], F32)
                nc.vector.tensor_tensor(out=yo, in0=yps, in1=bob, op=ALU.add)
                nc.sync.dma_start(out=y.ap()[it * P:(it + 1) * P, :], in_=yo)

            for it in range(4):
                y_proj(it)
                if tailn:
                    tailn.pop(0)()
            for it in range(4, NQ // P):
                y_proj(it)

    nc.compile()
    return nc


def _get_nc():
    global _BUILT
    if _BUILT is None:
        _BUILT = _build()
    return _BUILT


def prep_in_maps(inputs):
    x = np.asarray(inputs["x"], np.float32)
    B, N, D = x.shape
    assert (B, N, D) == (4, 2048, 512)

    gam = np.asarray(inputs["ln_gamma"], np.float64)
    bet = np.asarray(inputs["ln_beta"], np.float64)
    Wq = np.asarray(inputs["Wq"], np.float64)
    Wk = np.asarray(inputs["Wk"], np.float64)
    Wv = np.asarray(inputs["Wv"], np.float64)

    # fold gamma/beta:  (xn*gam+bet) @ W.T + b  ==  xn @ (W*gam).T + (b + W@bet)
    Wo = np.asarray(inputs["Wo"], np.float64)
    bq = np.asarray(inputs["bq"], np.float64) + Wq @ bet
    bv = np.asarray(inputs["bv"], np.float64) + Wv @ bet
    bo = np.asarray(inputs["bo"], np.float64) + Wo @ bv
    Wqg = Wq * gam[None, :]
    Wkg = Wk * gam[None, :]
    Wvg = Wv * gam[None, :]

    def cols(v):  # (512,) -> (128, 4): column t = v[128t:128(t+1)]
        return np.ascontiguousarray(
            np.asarray(v, np.float32).reshape(CC, P).T)

    def bcast(v):  # (512,) -> (128, 512)
        return np.ascontiguousarray(
            np.broadcast_to(np.asarray(v, np.float32), (P, DIM)))

    bf16 = ml_dtypes.bfloat16
    common = {
        "wqT": np.ascontiguousarray(Wqg.astype(np.float32).T.astype(bf16)),
        "wkT": np.ascontiguousarray(Wkg.astype(np.float32).T.astype(bf16)),
        "wvT": np.ascontiguousarray(Wvg.astype(np.float32).T.astype(bf16)),
        "woT": np.ascontiguousarray(
            np.asarray(inputs["Wo"], np.float32).T
            .reshape(H, DK, DIM).transpose(1, 0, 2).astype(bf16)),
        "qb_c": cols(bq),
        "bo_b": bcast(bo),
    }
    in_maps = []
    for c in range(N_CORES):
        b, half = divmod(c, 2)
        o = half * NQ
        xc = np.concatenate([x[b, o:o + NQ], x[b, NQ - o:N - o]], axis=0)
        in_maps.append({"xq": np.ascontiguousarray(xc), **common})
    return in_maps


def kernel(x, ln_gamma, ln_beta, Wq, bq, Wk, bk, Wv, bv, Wo, bo):
    in_maps = prep_in_maps(dict(
        x=x, ln_gamma=ln_gamma, ln_beta=ln_beta, Wq=Wq, bq=bq, Wk=Wk, bk=bk,
        Wv=Wv, bv=bv, Wo=Wo, bo=bo))

    nc = _get_nc()
    res = run_bass_kernel_spmd(nc, in_maps, core_ids=list(range(N_CORES)))

    B, N, D = 4, 2048, DIM
    out = np.empty((B, N, D), np.float32)
    for c in range(N_CORES):
        b, half = divmod(c, 2)
        o = half * NQ
        out[b, o:o + NQ] = res.results[c]["y"]
    return out
